# revision 44
# baseline (speedup 1.0000x reference)
"""K-means step kernel for Trainium2 (8 NeuronCores, data-parallel over n).

scores[n,k] = ||c_k||^2 - 2 x_n.c_k ; assign = argmin_k ; new centroids =
segment-mean.  Per core (n_loc = n/8 = 16384 rows, 128 tiles of 128):

  mm1 (PE, 6144 cyc/tile): dot' = x @ (-2C)^T to ~7e-4 absolute noise via
       T1 = fp16(x) @ fp16(-2C)          (2 chunk matmuls x 4 col groups)
       T2 = e5m2(xh*2^-5) @ e5m2(Cl*2^5) (fp8 DoubleRow, net scale 1)
       T3 = e5m2(xl*2^5) @ e5m2(Ch*2^-5) (fp8 DoubleRow, net scale 1)
       accumulated per 512-col fp32 PSUM group.  c_sq is folded into the
       first 1024 cols on the PE (fp16 hi/lo rows, 2-hot selector matmul,
       +1024 cyc) and added to the other 1024 cols on the DVE -- this
       balances PE against DVE (~3.85us/tile each).
  DVE: h0: fused tensor_scalar copy PSUM->SBUF + row-min accum; h1:
       tensor_tensor (+c_sq) then in-place tensor_scalar with min accum
       chained from h0's partial min via scalar2.
  Act: Sign(min - s) -> complement one-hot (-1 off-cluster, 0 at argmin)
       written straight to fp8 in [128, 2, 2048] pair layout.
  mm2 (PE, 2052 cyc/tile): acc_k -= sum_{n not in k} [x8_n, 1] via fp8
       DoubleRow matmuls (one-hot pair stationary, x_aug hi+lo moving).
       A group of 10 pairs accumulates in PSUM; its 16 output chunks are
       then spread evenly over the next group's tiles (burst emission
       serializes on the 2-buffer pc ring behind queued DVE work).
  Host: sums_k = total8 + acc_k, counts_k = n_loc + acc_k[:,256] per core,
       all-reduce over cores, divide, keep old centroid where empty.

  Measured: 508718 ns (CoreSim), rel err 1.09e-3 (baseline: 1117741 ns).
"""

import numpy as np

import concourse.bass as bass
import concourse.mybir as mybir
import concourse.tile as tile
from concourse.bass_utils import run_bass_kernel_spmd
from concourse.vector_clock import ScopedClock

# ---------------------------------------------------------------------------
# Workaround: walrus rejects >1 sem wait on CTRL (drain/nop) instructions.
# Split the TileContext exit-drain's waits across one NOP per wait.
_MAXW = 1


def _patched_drain_and_barrier(self, tick_clock, wait_clock):
    nc = self.nc
    drain_inst = nc.sync.drain()
    wait_clock.add_sem_waits(
        drain_inst.ins, ScopedClock({None: tick_clock.global_clock})
    )
    si = drain_inst.ins.sync_info
    waits = list(si.on_wait) if si and si.on_wait else []
    if len(waits) > _MAXW:
        drain_inst.ins.sync_info = mybir.SyncInfo(
            on_wait=waits[:_MAXW], on_update=list(si.on_update or [])
        )
        rest = waits[_MAXW:]
        for i in range(0, len(rest), _MAXW):
            nop = nc.sync.nop()
            nop.ins.sync_info = mybir.SyncInfo(
                on_wait=rest[i : i + _MAXW], on_update=[]
            )
    nc.all_engine_barrier()
    popped = nc._tile_sem_poison_stack.pop()
    assert popped is self._sem_poison
    nc.clear_and_free_semaphores(list(self.sems.allocated().values()))
    nc.all_engine_barrier()


tile.TileContext._drain_and_barrier = _patched_drain_and_barrier

# This walrus build accepts only ONE sync wait per instruction, but Tile's
# scheduler emits several on phase joins.  Rewrite the BIR before compiling:
# excess waits move onto same-engine NOPs inserted just before the
# instruction (identical semantics: all waits still complete before it).
import json as _json

import concourse.bass2jax as _bass2jax

_orig_compile_bir = _bass2jax.compile_bir_kernel


def _split_waits_compile(bir_json, tmpdir, neff_name="file.neff"):
    j = _json.loads(bir_json)
    cnt = 0
    for f in j["functions"]:
        for bb in f["blocks"]:
            out = []
            for ins in bb["instructions"]:
                si = ins.get("sync_info")
                ow = (si or {}).get("on_wait") or []
                if len(ow) > 1:
                    for w in ow[:-1]:
                        cnt += 1
                        out.append(
                            {
                                "debug": ins.get("debug"),
                                "engine": ins["engine"],
                                "ins": [],
                                "outs": [],
                                "name": f"I-wsplit-{cnt}",
                                "opcode": "NoOp",
                                "sync_info": {"on_update": [], "on_wait": [w]},
                            }
                        )
                    si["on_wait"] = [ow[-1]]
                out.append(ins)
            bb["instructions"] = out
    return _orig_compile_bir(_json.dumps(j).encode(), tmpdir, neff_name=neff_name)


_bass2jax.compile_bir_kernel = _split_waits_compile
# ---------------------------------------------------------------------------

N_CORES = 8
P = 128
F16 = mybir.dt.float16
F32 = mybir.dt.float32
E4 = mybir.dt.float8e4
E5 = mybir.dt.float8e5
DR = mybir.MatmulPerfMode.DoubleRow
ADD = mybir.AluOpType.add
MIN = mybir.AluOpType.min
SIGN = mybir.ActivationFunctionType.Sign

_KERNEL_CACHE = {}


def build_kernel(n_loc, k, d, group=10):
    ntiles = n_loc // P          # 128
    npairs = ntiles // 2         # 64
    daug = d + 1                 # 257
    nq = k // 512                # 4 col groups for T1
    nchunks = k // P             # 16 output chunks for mm2

    nc = bass.Bass()
    xhT = nc.declare_dram_parameter("xhT", [d, n_loc], F16, isOutput=False)
    x5 = nc.declare_dram_parameter("x5", [P, 2, n_loc], E5, isOutput=False)
    xl5 = nc.declare_dram_parameter("xl5", [P, 2, n_loc], E5, isOutput=False)
    ch16 = nc.declare_dram_parameter("ch16", [d, k], F16, isOutput=False)
    cl5 = nc.declare_dram_parameter("cl5", [P, 2, k], E5, isOutput=False)
    ch5 = nc.declare_dram_parameter("ch5", [P, 2, k], E5, isOutput=False)
    csqb = nc.declare_dram_parameter("csqb", [P, k // 2], F32, isOutput=False)
    foldsel = nc.declare_dram_parameter("foldsel", [P, P], F16, isOutput=False)
    bhl = nc.declare_dram_parameter("bhl", [P, k // 2], F16, isOutput=False)
    xa8h = nc.declare_dram_parameter("xa8h", [P, 2, npairs * daug], E4, isOutput=False)
    xa8l = nc.declare_dram_parameter("xa8l", [P, 2, npairs * daug], E4, isOutput=False)
    out = nc.declare_dram_parameter("out", [k, daug], F32, isOutput=True)

    with tile.TileContext(nc) as tc:
        with (
            tc.tile_pool(name="consts", bufs=1) as consts,
            tc.tile_pool(name="xt", bufs=3) as xtp,
            tc.tile_pool(name="xa", bufs=2 * group + 2) as xap,
            tc.tile_pool(name="oh", bufs=2 * group + 2) as ohp,
            tc.tile_pool(name="sc", bufs=3) as scp,
            tc.tile_pool(name="mp", bufs=3) as mp,
            tc.tile_pool(name="ps1", bufs=3, space="PSUM") as ps1,
            tc.tile_pool(name="ps2", bufs=2, space="PSUM") as ps2,
        ):
            cht = consts.tile([P, 2, k], F16, tag="cht", name="cht")
            ch16r = ch16[:, :].rearrange("(two p) k -> p two k", two=2)
            for j in range(4):
                js = slice(j * (k // 4), (j + 1) * (k // 4))
                nc.sync.dma_start(out=cht[:, :, js], in_=ch16r[:, :, js])
            cl5t = consts.tile([P, 2, k], E5, tag="cl5t", name="cl5t")
            ch5t = consts.tile([P, 2, k], E5, tag="ch5t", name="ch5t")
            for j in range(2):
                js = slice(j * (k // 2), (j + 1) * (k // 2))
                nc.sync.dma_start(out=cl5t[:, :, js], in_=cl5[:, :, js])
                nc.sync.dma_start(out=ch5t[:, :, js], in_=ch5[:, :, js])
            csq = consts.tile([P, k // 2], F32, tag="csq", name="csq")
            nc.sync.dma_start(out=csq, in_=csqb[:, :])
            fsel = consts.tile([P, P], F16, tag="fsel", name="fsel")
            nc.sync.dma_start(out=fsel, in_=foldsel[:, :])
            bht = consts.tile([P, k // 2], F16, tag="bht", name="bht")
            nc.sync.dma_start(out=bht, in_=bhl[:, :])
            acc = consts.tile([P, nchunks * daug], F32, tag="acc", name="acc")
            nc.vector.memset(acc, 0.0)

            def emit_mm2_chunks(pend, c_lo, c_hi):
                ohs, xhs, xls = pend
                ng = len(ohs)
                for c in range(c_lo, c_hi):
                    pc = ps2.tile([P, daug], F32, tag="ps2", name="pc")
                    csl = slice(c * P, (c + 1) * P)
                    for g in range(ng):
                        st = ohs[g][:, :, csl]
                        nc.tensor.matmul(
                            pc[:, 0:d], st, xhs[g][:, :, 0:d],
                            start=(g == 0), stop=False, perf_mode=DR,
                        )
                        nc.tensor.matmul(
                            pc[:, 0:d], st, xls[g][:, :, 0:d],
                            start=False, stop=(g == ng - 1), perf_mode=DR,
                        )
                    for g in range(ng):
                        nc.tensor.matmul(
                            pc[:, d:daug], ohs[g][:, :, csl], xhs[g][:, :, d:daug],
                            start=(g == 0), stop=(g == ng - 1), perf_mode=DR,
                        )
                    nc.vector.tensor_tensor(
                        acc[:, c * daug : (c + 1) * daug], pc,
                        acc[:, c * daug : (c + 1) * daug], op=ADD,
                    )

            def emit_mm2(pend):
                emit_mm2_chunks(pend, 0, nchunks)

            pending = None
            next_chunk = 0   # next un-emitted chunk of `pending`
            win_start = 0    # tile index where `pending` was set
            win_tiles = 1    # tiles available to spread `pending` over
            cur = ([], [], [])
            oh_t = None
            nfull = (npairs - 4) // group
            gsizes = [group] * nfull + [npairs - 4 - nfull * group, 4]
            gsizes = [g for g in gsizes if g > 0]
            gidx = 0
            for i in range(ntiles):
                xht = [xtp.tile([P, P], F16, tag=f"xh{j}", name=f"xh{j}") for j in range(2)]
                for j in range(2):
                    nc.sync.dma_start(
                        out=xht[j], in_=xhT[j * P : (j + 1) * P, i * P : (i + 1) * P]
                    )
                x5t = xtp.tile([P, 2, P], E5, tag="x5t", name="x5t")
                xl5t = xtp.tile([P, 2, P], E5, tag="xl5t", name="xl5t")
                nc.sync.dma_start(out=x5t, in_=x5[:, :, i * P : (i + 1) * P])
                nc.sync.dma_start(out=xl5t, in_=xl5[:, :, i * P : (i + 1) * P])
                if i % 2 == 0:
                    p = i // 2
                    xh8 = xap.tile([P, 2, daug], E4, tag="xa8h", name="xh8")
                    xl8 = xap.tile([P, 2, daug], E4, tag="xa8l", name="xl8")
                    nc.sync.dma_start(
                        out=xh8, in_=xa8h[:, :, p * daug : (p + 1) * daug]
                    )
                    nc.sync.dma_start(
                        out=xl8, in_=xa8l[:, :, p * daug : (p + 1) * daug]
                    )
                    oh_t = ohp.tile([P, 2, k], E4, tag="oh", name="oh_t")

                scores = scp.tile([P, k // 2], F32, tag="scores", name="scores")
                s0t = scp.tile([P, k // 2], F32, tag="scores0", name="s0t")
                ma = mp.tile([P, 1], F32, tag="ma", name="ma")
                m1 = mp.tile([P, 1], F32, tag="m1", name="m1")
                for h in range(2):
                    ph = ps1.tile([P, 1024], F32, tag="ps1", name="ph")
                    for q in range(2):
                        col = h * 1024 + q * 512
                        qs = slice(q * 512, (q + 1) * 512)
                        nc.tensor.matmul(
                            ph[:, qs], xht[0], cht[:, 0, col : col + 512],
                            start=True, stop=False,
                        )
                        nc.tensor.matmul(
                            ph[:, qs], xht[1], cht[:, 1, col : col + 512],
                            start=False, stop=False,
                        )
                        if h == 0:
                            # fold csq (fp16 hi/lo rows) into the h0 psum
                            nc.tensor.matmul(
                                ph[:, qs], fsel, bht[:, col : col + 512],
                                start=False, stop=False,
                            )
                        for g in (2 * q, 2 * q + 1):
                            col2 = h * 1024 + g * 256
                            gs = slice(g * 256, (g + 1) * 256)
                            nc.tensor.matmul(
                                ph[:, gs], x5t, cl5t[:, :, col2 : col2 + 256],
                                start=False, stop=False, perf_mode=DR,
                            )
                            nc.tensor.matmul(
                                ph[:, gs], xl5t, ch5t[:, :, col2 : col2 + 256],
                                start=False, stop=(g == 2 * q + 1), perf_mode=DR,
                            )
                    if h == 0:
                        # fused copy PSUM->SBUF + row-min (frees the psum half)
                        nc.vector.tensor_scalar(
                            out=s0t, in0=ph, scalar1=0.0, scalar2=None,
                            op0=ADD, op1=MIN, accum_out=ma,
                        )
                    else:
                        nc.vector.tensor_tensor(scores, ph, csq, op=ADD)
                        # in-place identity + min accum chained from ma
                        nc.vector.tensor_scalar(
                            out=scores, in0=scores, scalar1=0.0, scalar2=ma,
                            op0=ADD, op1=MIN, accum_out=m1,
                        )
                nc.scalar.activation(
                    oh_t[:, i % 2, 0:1024], s0t, SIGN, bias=m1, scale=-1.0
                )
                nc.scalar.activation(
                    oh_t[:, i % 2, 1024:2048], scores, SIGN, bias=m1, scale=-1.0
                )

                if pending is not None and next_chunk < nchunks:
                    target = min(nchunks, (nchunks * (i - win_start + 1)) // win_tiles)
                    if target > next_chunk:
                        emit_mm2_chunks(pending, next_chunk, target)
                        next_chunk = target
                if i % 2 == 1:
                    cur[0].append(oh_t)
                    cur[1].append(xh8)
                    cur[2].append(xl8)
                    if len(cur[0]) == gsizes[gidx]:
                        if pending is not None and next_chunk < nchunks:
                            emit_mm2_chunks(pending, next_chunk, nchunks)
                        pending = cur
                        next_chunk = 0
                        win_start = i + 1
                        gidx += 1
                        win_tiles = 2 * gsizes[min(gidx, len(gsizes) - 1)]
                        cur = ([], [], [])
            if pending is not None and next_chunk < nchunks:
                emit_mm2_chunks(pending, next_chunk, nchunks)
            if cur[0]:
                emit_mm2(cur)

            for c in range(nchunks):
                nc.sync.dma_start(
                    out=out[c * P : (c + 1) * P, :],
                    in_=acc[:, c * daug : (c + 1) * daug],
                )
    return nc


def _np_dt(dt):
    return mybir.dt.np(dt)


def _prep_inputs(x, C):
    n, d = x.shape
    k = C.shape[0]
    n_loc = n // N_CORES
    npairs = n_loc // (2 * P)
    daug = d + 1
    e4 = _np_dt(E4)
    e5 = _np_dt(E5)

    x64 = x.astype(np.float64)
    C64 = C.astype(np.float64)
    Cp = -2.0 * C64
    csq = np.sum(C64 * C64, axis=1).astype(np.float32)

    xh16 = x64.astype(np.float16)
    xl = x64 - xh16.astype(np.float64)
    Ch16 = Cp.astype(np.float16)
    Cl = Cp - Ch16.astype(np.float64)

    x5m = (xh16.astype(np.float64) * 2.0**-5).astype(e5)     # [n, d]
    xl5m = (xl * 2.0**5).astype(e5)
    cl5m = (Cl * 2.0**5).astype(e5)                           # [k, d]
    ch5m = (Ch16.astype(np.float64) * 2.0**-5).astype(e5)

    xa_h = x64.astype(e4)                                     # [n, d]
    xa_l = (x64 - xa_h.astype(np.float64)).astype(e4)
    x8sum = xa_h.astype(np.float64) + xa_l.astype(np.float64)  # device-visible x

    def pack_kd(a):  # [k_or_n, d] -> [128, 2, k_or_n] d-chunk pack
        return np.ascontiguousarray(a.T.reshape(2, P, -1).transpose(1, 0, 2))

    ch16T = np.ascontiguousarray(Ch16.T)                      # [d, k]
    cl5p = pack_kd(cl5m)
    ch5p = pack_kd(ch5m)
    kh = k // 2
    csqb = np.ascontiguousarray(np.broadcast_to(csq[kh:], (P, kh)))
    foldsel = np.zeros((P, P), np.float16)
    foldsel[0, :] = 1.0
    foldsel[1, :] = 1.0
    bh = csq[:kh].astype(np.float64)
    bhi = bh.astype(np.float16)
    blo = (bh - bhi.astype(np.float64)).astype(np.float16)
    bhl = np.zeros((P, kh), np.float16)
    bhl[0, :] = bhi
    bhl[1, :] = blo

    ones = np.ones((n_loc, 1), np.float64)
    zeros = np.zeros((n_loc, 1), np.float64)

    in_maps = []
    totals = []
    for c in range(N_CORES):
        sl = slice(c * n_loc, (c + 1) * n_loc)
        xa_hc = np.concatenate([xa_h[sl].astype(np.float64), ones], 1).astype(e4)
        xa_lc = np.concatenate([xa_l[sl].astype(np.float64), zeros], 1).astype(e4)

        def pack_pairs(a):  # [n_loc, daug] -> [128, 2, npairs*daug]
            return np.ascontiguousarray(
                a.reshape(npairs, 2, P, daug)
                .transpose(2, 1, 0, 3)
                .reshape(P, 2, npairs * daug)
            )

        in_maps.append(
            {
                "xhT": np.ascontiguousarray(xh16[sl].T),
                "x5": pack_kd(x5m[sl]),
                "xl5": pack_kd(xl5m[sl]),
                "ch16": ch16T,
                "cl5": cl5p,
                "ch5": ch5p,
                "csqb": csqb,
                "foldsel": foldsel,
                "bhl": bhl,
                "xa8h": pack_pairs(xa_hc),
                "xa8l": pack_pairs(xa_lc),
            }
        )
        totals.append(x8sum[sl].sum(axis=0))
    return in_maps, totals


def kernel(x, centroids, _trace=False):
    x = np.asarray(x, dtype=np.float32)
    C = np.asarray(centroids, dtype=np.float32)
    n, d = x.shape
    k = C.shape[0]
    n_loc = n // N_CORES

    key = (n_loc, k, d)
    if key not in _KERNEL_CACHE:
        _KERNEL_CACHE[key] = build_kernel(n_loc, k, d)
    nc = _KERNEL_CACHE[key]

    in_maps, totals = _prep_inputs(x, C)
    res = run_bass_kernel_spmd(
        nc, in_maps, core_ids=list(range(N_CORES)), trace=_trace
    )

    sums = np.zeros((k, d), np.float64)
    counts = np.zeros((k,), np.float64)
    for c in range(N_CORES):
        acc = res.results[c]["out"].astype(np.float64)  # [k, d+1], complement (-1)
        sums += totals[c][None, :] + acc[:, :d]
        counts += n_loc + acc[:, d]
    means = (sums / np.maximum(counts, 1.0)[:, None]).astype(np.float32)
    out = np.where(counts[:, None] > 0.5, means, C)
    if _trace:
        kernel._last_result = res
    return out.astype(np.float32)
